# revision 12
# baseline (speedup 1.0000x reference)
"""Expert-parallel MoE SwiGLU kernel for Trainium2 (8 NeuronCores).

Strategy: each of the 8 cores owns one expert's weights (w1/w3/w2).  Token
routing (the "all-to-all dispatch") is done host-side: tokens are gathered
per expert and padded to a fixed capacity T=480, and each core computes

    y_e = (silu(x_e @ w1_e) * (x_e @ w3_e)) @ w2_e          # [T, H]

for its expert's token set.  The host scatter-adds the weighted per-expert
outputs back into the [B, H] result.  The rare overflow tokens (experts
with more than T assigned tokens) are computed host-side in fp32.

All matmul operands are bf16 (PSUM accumulation stays fp32), which halves
HBM weight traffic vs fp32 and keeps the PE at 1 cycle/row for any moving
size.  Weights are host-repacked into PE-ready tiled layouts so every DMA
line is 4-8KB contiguous per partition.
"""

import numpy as np

_P = 128
_E = 8   # experts == cores
# fixed device token capacity per expert: B*K/E pairs minus expected top-k
# duplicate merges lands at ~480; the rare overflow tokens of hotter experts
# are computed host-side in exact fp32
_T = 480

_PROG_CACHE = {}
# test hooks: set TRACE=True before calling kernel() to capture an NTFF
# profile; the BassKernelResults of the last run lands in LAST_RUN.
TRACE = False
LAST_RUN = None


def _build_program(H, I):
    import concourse.bass as bass
    import concourse.tile as tile
    from concourse import bacc, mybir

    f32 = mybir.dt.float32
    bf16 = mybir.dt.bfloat16
    Silu = mybir.ActivationFunctionType.Silu
    ts = bass.ts

    T = _T
    HC = H // _P   # 16 contraction blocks for phase 1
    IC = I // _P   # 32 contraction blocks for phase 2
    assert H % _P == 0 and I % _P == 0

    nc = bacc.Bacc(
        "TRN2",
        target_bir_lowering=False,
        debug=False,
        enable_asserts=False,
        num_devices=_E,
    )
    # Host-pretiled layouts (per-partition lines are fully contiguous):
    #   x  [P, HC, T]     x[p, hc, t]  = x_tok[t, hc*P + p]          (bf16)
    #   w1 [IC*P, HC, P]  w1[ic*P+p, hc, j] = w1[hc*P+p, ic*P+j]     (bf16)
    #   w3 same as w1
    #   w2 [HC*P, IC, P]  w2[hc*P+p, ic, j] = w2[ic*P+p, hc*P+j]     (bf16)
    #   y  [H, T]         y[h, t] (fp32), h on partitions per block
    x = nc.dram_tensor("x", [_P, HC, T], bf16, kind="ExternalInput").ap()
    w1 = nc.dram_tensor("w1", [IC * _P, HC, _P], bf16, kind="ExternalInput").ap()
    w3 = nc.dram_tensor("w3", [IC * _P, HC, _P], bf16, kind="ExternalInput").ap()
    w2 = nc.dram_tensor("w2", [HC * _P, IC, _P], bf16, kind="ExternalInput").ap()
    y = nc.dram_tensor("y", [H, T], f32, kind="ExternalOutput").ap()

    NXC = 4           # x loads in 4 chunks so the first matmuls start early
    XW = HC // NXC

    with tile.TileContext(nc) as tc:
        with (
            tc.tile_pool(name="xp", bufs=1) as xp,
            tc.tile_pool(name="cp", bufs=1) as cp,
            tc.tile_pool(name="wp", bufs=8) as wp,
            tc.tile_pool(name="w2p", bufs=4) as w2p,
            tc.tile_pool(name="hp", bufs=1) as hp,
            tc.tile_pool(name="sp", bufs=2) as sp,
            tc.tile_pool(name="op", bufs=4) as op,
            tc.tile_pool(name="pp", bufs=7, space="PSUM") as pp,
            tc.tile_pool(name="wup", bufs=1, space="PSUM") as wup,
        ):
            zbias = cp.tile([_P, 1], f32)
            nc.any.memset(zbias[:], 0.0)

            # warm up the PE p-state during the initial x/w1 DMA wait with
            # throwaway matmuls on memset tiles, so the first real matmuls
            # run at full clock
            wl = cp.tile([_P, _P], bf16, tag="wl")
            nc.vector.memset(wl[:], 0.0)
            wr = cp.tile([_P, T], bf16, tag="wr")
            nc.vector.memset(wr[:], 0.0)
            wud = wup.tile([_P, T], f32, tag="wu")
            for _ in range(10):
                nc.tensor.matmul(
                    wud[:], lhsT=wl[:], rhs=wr[:], start=True, stop=True
                )

            # startup critical path: w1[0] leads the sync queue while x0
            # leads scalar, so the first gate matmuls start as early as
            # possible; remaining x chunks and w3[0] follow right behind
            w1s0 = wp.tile([_P, HC, _P], bf16, tag="w1", name="w1_0")
            nc.sync.dma_start(w1s0[:], w1[ts(0, _P), :, :])
            xcs = []
            w3s0 = None
            for c in range(NXC):
                xc = xp.tile([_P, XW, T], bf16, tag=f"x{c}", name=f"x_{c}")
                eng = (nc.scalar, nc.sync, nc.scalar, nc.sync)[c]
                eng.dma_start(xc[:], x[:, c * XW : (c + 1) * XW, :])
                xcs.append(xc)
                if c == 0:
                    # w3[0] rides second on the scalar queue (before x2):
                    # the ic=0 up-group needs it right after the gate-group,
                    # well before x2/x3 finish pacing the gate matmuls
                    w3s0 = wp.tile([_P, HC, _P], bf16, tag="w3", name="w3_0")
                    nc.scalar.dma_start(w3s0[:], w3[ts(0, _P), :, :])

            def xsl(hc):
                return xcs[hc // XW][:, hc % XW, :]

            # resident h^T [p(i), ic, t] in bf16
            hs = hp.tile([_P, IC, T], bf16)

            # ---- phase 1: h^T[i, t] = silu(w1^T x)[i, t] * (w3^T x)[i, t]
            for ic in range(IC):
                if ic == 0:
                    w1s, w3s = w1s0, w3s0
                else:
                    w1s = wp.tile([_P, HC, _P], bf16, tag="w1", name=f"w1_{ic}")
                    nc.sync.dma_start(w1s[:], w1[ts(ic, _P), :, :])
                    w3s = wp.tile([_P, HC, _P], bf16, tag="w3", name=f"w3_{ic}")
                    nc.scalar.dma_start(w3s[:], w3[ts(ic, _P), :, :])

                pg = pp.tile([_P, T], f32, tag="ps", name=f"pg_{ic}")
                pu = pp.tile([_P, T], f32, tag="ps", name=f"pu_{ic}")
                for hc in range(HC):
                    nc.tensor.matmul(
                        pg[:],
                        lhsT=w1s[:, hc, :],
                        rhs=xsl(hc),
                        start=(hc == 0),
                        stop=(hc == HC - 1),
                    )
                for hc in range(HC):
                    nc.tensor.matmul(
                        pu[:],
                        lhsT=w3s[:, hc, :],
                        rhs=xsl(hc),
                        start=(hc == 0),
                        stop=(hc == HC - 1),
                    )
                sg = sp.tile([_P, T], f32, tag="sg", name=f"sg_{ic}")
                nc.scalar.activation(sg[:], pg[:], Silu, bias=zbias[:])
                nc.vector.tensor_mul(hs[:, ic, :], sg[:], pu[:])

            # ---- phase 2: y[h, t] = sum_i w2[i, h] * h^T[i, t]
            # w2 tiles stream on both queues; pool lookahead (bufs=4) makes
            # the first tiles prefetch during phase 1's tail.
            for hc2 in range(HC):
                w2s = w2p.tile([_P, IC, _P], bf16, tag="w2", name=f"w2_{hc2}")
                w2_eng = nc.sync if hc2 % 2 == 0 else nc.scalar
                w2_eng.dma_start(w2s[:], w2[ts(hc2, _P), :, :])

                # the last block accumulates its column halves as separate
                # PSUM groups so the first half's writeback overlaps the
                # second half's matmuls, shortening the drain
                TH = T // 2
                col_groups = (
                    [slice(0, T)] if hc2 < HC - 1 else [slice(0, TH), slice(TH, T)]
                )
                py = pp.tile([_P, T], f32, tag="ps", name=f"py_{hc2}")
                ot = op.tile([_P, T], f32, tag="ot", name=f"ot_{hc2}")
                for cg in col_groups:
                    for ic in range(IC):
                        nc.tensor.matmul(
                            py[:, cg],
                            lhsT=w2s[:, ic, :],
                            rhs=hs[:, ic, cg],
                            start=(ic == 0),
                            stop=(ic == IC - 1),
                        )
                    # write back in half-column chunks on both queues so
                    # every block's drain is pipelined
                    for half in ([0, 1] if cg.stop - cg.start == T else [0]):
                        sl = (
                            slice(half * TH, (half + 1) * TH)
                            if cg.stop - cg.start == T
                            else cg
                        )
                        nc.vector.tensor_copy(ot[:, sl], py[:, sl])
                        out_eng = nc.scalar if (hc2 + half) % 2 == 0 else nc.sync
                        out_eng.dma_start(y[ts(hc2, _P), sl], ot[:, sl])

    nc.compile()
    return nc


def _get_program(H, I):
    key = (H, I)
    if key not in _PROG_CACHE:
        _PROG_CACHE[key] = _build_program(H, I)
    return _PROG_CACHE[key]


def kernel(x, expert_indices, expert_weights, w1, w2, w3):
    global LAST_RUN
    import ml_dtypes
    from concourse.bass_utils import run_bass_kernel_spmd

    bf16 = ml_dtypes.bfloat16
    x = np.ascontiguousarray(np.asarray(x, dtype=np.float32))
    idx = np.asarray(expert_indices).astype(np.int64)
    wts = np.asarray(expert_weights, dtype=np.float32)
    w1 = np.asarray(w1, dtype=np.float32)
    w2 = np.asarray(w2, dtype=np.float32)
    w3 = np.asarray(w3, dtype=np.float32)

    B, H = x.shape
    E, _, I = w1.shape
    assert E == _E, f"expected {_E} experts, got {E}"
    HC, IC, P, T = H // _P, I // _P, _P, _T

    # host-side dispatch: per-token expert weight matrix (merges duplicate
    # top-k hits of the same expert), then token lists per expert
    wmat = np.zeros((B, E), np.float32)
    np.add.at(wmat, (np.arange(B)[:, None], idx), wts)
    sel = np.zeros((B, E), bool)
    sel[np.arange(B)[:, None], idx] = True
    toks = [np.nonzero(sel[:, e])[0] for e in range(E)]

    nc = _get_program(H, I)

    xb = x.astype(bf16)
    in_maps = []
    for e in range(E):
        te = toks[e][:T]
        n = len(te)
        xe = np.zeros((P, HC, T), bf16)
        if n:
            xe[:, :, :n] = xb[te].reshape(n, HC, P).transpose(2, 1, 0)
        w1e = w1[e].astype(bf16).reshape(HC, P, IC, P).transpose(2, 1, 0, 3)
        w3e = w3[e].astype(bf16).reshape(HC, P, IC, P).transpose(2, 1, 0, 3)
        w2e = w2[e].astype(bf16).reshape(IC, P, HC, P).transpose(2, 1, 0, 3)
        in_maps.append(
            {
                "x": np.ascontiguousarray(xe),
                "w1": np.ascontiguousarray(w1e).reshape(IC * P, HC, P),
                "w3": np.ascontiguousarray(w3e).reshape(IC * P, HC, P),
                "w2": np.ascontiguousarray(w2e).reshape(HC * P, IC, P),
            }
        )
    res = run_bass_kernel_spmd(nc, in_maps, list(range(_E)), trace=TRACE)
    LAST_RUN = res

    out = np.zeros((B, H), np.float32)
    for e in range(E):
        te = toks[e][:T]
        if len(te):
            ye = res.results[e]["y"][:, : len(te)].T  # [H, T] -> [n, H]
            out[te] += wmat[te, e][:, None] * ye
        ov = toks[e][T:]  # overflow tokens: exact fp32 on host
        if len(ov):
            xo = x[ov]
            g = xo @ w1[e]
            u = xo @ w3[e]
            h = (g / (1.0 + np.exp(-g))) * u
            out[ov] += wmat[ov, e][:, None] * (h @ w2[e])
    return out
